# revision 11
# baseline (speedup 1.0000x reference)
"""CausalSelfAttention (depthwise-conv + RoPE + causal SDPA + proj) on 8 Trainium2 cores.

v4: the per-call host->device traffic dominates this dispatch path, so each
core ships ONE consolidated fp16 blob: its 512-token x chunk (transposed,
2-col conv halo), a 1/8 shard of the RoPE tables, its 2 heads' QKV / proj
weight slices, and the conv taps. A device-side AllGather rebuilds the full
sequence (+ full RoPE tables) on every core; compute is tensor-parallel over
heads in fp16 with fp32 PSUM accumulation; the output projection partials are
ReduceScattered per chunk so each core returns a [256, 4096] fp16 slice of
the transposed output. Identity / ones / causal masks are generated on device
(memset + affine_select)."""
import sys
sys.path.insert(0, '/opt/trn_rl_repo')
import numpy as np
import concourse.bass as bass
import concourse.mybir as mybir
import concourse.tile as tile
from concourse import bacc
from concourse import bass_utils
from concourse import masks as bmasks

F32 = mybir.dt.float32
F16 = mybir.dt.float16
AF = mybir.ActivationFunctionType
OP = mybir.AluOpType

B, T, C = 2, 2048, 2048
H, D = 16, 128
NC = 8
HPC = H // NC          # heads per core = 2
CHW = 512              # chunk width (tokens)
NCH = T // CHW         # chunks per batch = 4
TCH = B * NCH          # total chunks = 8
CT = C // 128          # 16 c-tiles
STW = 128              # s-tile width
XW = CHW + 2           # x chunk width incl. 2-col conv halo

# blob layout, all fp16, width 256 (row counts in 256-el blob rows)
XROWS = C * XW // 256          # 4112: x chunk [2048, 514]
TABROWS = 128                  # [16, 2048] table shard (cos 0:64, sin 64:128 of full)
AGROWS = XROWS + TABROWS       # 4240: AllGathered prefix
QKV0 = AGROWS                  # qkv [6144, 256]
WO0 = QKV0 + 3 * C             # 10384: wo [256, 2048] -> 2048 rows
CW0 = WO0 + C                  # 12432: convw [2048, 3] fp16 -> 24 rows
NROWS = CW0 + C * 3 // 256     # 12456


def build_program():
    nc = bacc.Bacc("TRN2", target_bir_lowering=False, debug=False, num_devices=NC)

    blob_d = nc.dram_tensor("blob", [NROWS, 256], F16, kind="ExternalInput").ap()
    outp_d = nc.dram_tensor("outp", [HPC * D, B * T], F16, kind="ExternalOutput").ap()

    from contextlib import ExitStack
    with tile.TileContext(nc) as tc:
        with ExitStack() as stack:
            dram = stack.enter_context(tc.tile_pool(name="dram", bufs=1, space="DRAM"))
            drp = stack.enter_context(tc.tile_pool(name="drp", bufs=2, space="DRAM"))
            wr = stack.enter_context(tc.tile_pool(name="wr", bufs=1))
            cst = stack.enter_context(tc.tile_pool(name="const", bufs=1))
            xtp = stack.enter_context(tc.tile_pool(name="xt", bufs=2))
            xcp = stack.enter_context(tc.tile_pool(name="xc", bufs=1))
            tmp = stack.enter_context(tc.tile_pool(name="tmp", bufs=2))
            rpp = stack.enter_context(tc.tile_pool(name="rp", bufs=2))
            rdp = stack.enter_context(tc.tile_pool(name="rd", bufs=2))
            qsp = stack.enter_context(tc.tile_pool(name="qs", bufs=1))
            kvp = stack.enter_context(tc.tile_pool(name="kv", bufs=1))
            vstgp = stack.enter_context(tc.tile_pool(name="vstg", bufs=2))
            ep = stack.enter_context(tc.tile_pool(name="e", bufs=3))
            yp = stack.enter_context(tc.tile_pool(name="y", bufs=1))
            obp = stack.enter_context(tc.tile_pool(name="ob", bufs=4))
            pmm = stack.enter_context(tc.tile_pool(name="pmm", bufs=2, space="PSUM"))
            pS = stack.enter_context(tc.tile_pool(name="pS", bufs=2, space="PSUM"))
            pU = stack.enter_context(tc.tile_pool(name="pU", bufs=2, space="PSUM"))
            pD = stack.enter_context(tc.tile_pool(name="pD", bufs=2, space="PSUM"))

            # ---- AllGather x chunks + table shards: full sequence everywhere ----
            ag_in = dram.tile([AGROWS, 256], F16, name="ag_in")
            nc.gpsimd.dma_start(ag_in[:], blob_d[0:AGROWS, :])
            xg = dram.tile([NC * AGROWS, 256], F16, name="xg")
            nc.gpsimd.collective_compute(
                "AllGather", mybir.AluOpType.bypass,
                replica_groups=[list(range(NC))],
                ins=[ag_in.opt()], outs=[xg.opt()],
            )
            xg_t = xg[:].tensor

            # ---- RoPE tables: collect the 8 shards, upconvert to fp32 ----
            tabs16 = cst.tile([128, T], F16, tag="tabs16")
            for r in range(NC):
                nc.sync.dma_start(
                    tabs16[r * 16:(r + 1) * 16, :],
                    bass.AP(tensor=xg_t, offset=(r * AGROWS + XROWS) * 256,
                            ap=[[2048, 16], [1, 2048]]))
            tabs32 = cst.tile([128, T], F32, tag="tabs32")
            nc.vector.tensor_copy(tabs32[:], tabs16[:])
            cosT = tabs32[0:64, :]         # cos, rows j=0..63
            sinT = tabs32[64:128, :]       # sin, rows j=0..63 (at partitions 64+)

            # ---- generated constants ----
            ident = cst.tile([128, 128], F16, tag="ident")
            bmasks.make_identity(nc, ident[:])
            ones16 = cst.tile([128, 128], F16, tag="ones16")
            nc.gpsimd.memset(ones16[:], 1.0)
            cw = []
            for ct in range(CT):
                c16 = tmp.tile([128, 3], F16, tag="cw16")
                nc.sync.dma_start(
                    c16[:],
                    bass.AP(tensor=blob_d.tensor, offset=CW0 * 256 + ct * 128 * 3,
                            ap=[[3, 128], [1, 3]]))
                t_ = cst.tile([128, 3], F32, tag=f"cw{ct}")
                nc.vector.tensor_copy(t_[:], c16[:])
                cw.append(t_)

            # ---- weights resident in SBUF (fp16, used directly by PE) ----
            wq_r, wk_r, wv_r = [], [], []
            for qi, dst in ((0, wq_r), (1, wk_r), (2, wv_r)):
                for ct in range(CT):
                    t_ = wr.tile([128, HPC * D], F16, tag=f"w{qi}r{ct}")
                    r0 = QKV0 + qi * C + ct * 128
                    nc.sync.dma_start(t_[:], blob_d[r0:r0 + 128, :])
                    dst.append(t_)
            wo_r = []
            for hi in range(HPC):
                t_ = wr.tile([128, C], F16, tag=f"wor{hi}")
                nc.sync.dma_start(
                    t_[:],
                    bass.AP(tensor=blob_d.tensor, offset=(WO0 + hi * 1024) * 256,
                            ap=[[2048, 128], [1, 2048]]))
                wo_r.append(t_)

            # ---- main loop over the 8 chunks (b major, ch minor) ----
            for b in range(B):
                k_all = [kvp.tile([D, T], F16, tag=f"k{h}", name=f"kall{b}_{h}") for h in range(HPC)]
                v_all = [kvp.tile([128, T], F16, tag=f"v{h}", name=f"vall{b}_{h}") for h in range(HPC)]
                for ch in range(NCH):
                    g = b * NCH + ch               # global chunk id / xg block
                    t0 = ch * CHW                  # within-batch t offset
                    # ---- load + depthwise causal conv ----
                    xc = []
                    for ct in range(CT):
                        xt = xtp.tile([128, XW], F16, tag="xt")
                        nc.sync.dma_start(
                            xt[:],
                            bass.AP(tensor=xg_t, offset=g * AGROWS * 256 + ct * 128 * XW,
                                    ap=[[XW, 128], [1, XW]]))
                        ta = tmp.tile([128, CHW], F16, tag="t1")
                        nc.scalar.mul(ta[:], xt[:, 0:CHW], cw[ct][:, 0:1])
                        tb = tmp.tile([128, CHW], F16, tag="t2")
                        nc.vector.scalar_tensor_tensor(tb[:], xt[:, 1:CHW + 1], cw[ct][:, 1:2], ta[:], OP.mult, OP.add)
                        xct = xcp.tile([128, CHW], F16, tag=f"xc{ct}")
                        nc.vector.scalar_tensor_tensor(xct[:], xt[:, 2:CHW + 2], cw[ct][:, 2:3], tb[:], OP.mult, OP.add)
                        xc.append(xct)

                    # ---- QKV + rope ----
                    q_sb = []
                    for h in range(HPC):
                        hs = slice(h * D, (h + 1) * D)
                        cs = slice(t0, t0 + CHW)
                        # q
                        q_ps = pmm.tile([128, CHW], F32, tag="mm")
                        for ct in range(CT):
                            nc.tensor.matmul(q_ps[:], wq_r[ct][:, hs], xc[ct][:],
                                             start=(ct == 0), stop=(ct == CT - 1))
                        qt = qsp.tile([128, CHW], F16, tag=f"q{h}")
                        at = rpp.tile([64, CHW], F16, tag="ra")
                        nc.vector.tensor_tensor(at[:], q_ps[0:64, :], cosT[:, cs], OP.mult)
                        mt = rpp.tile([64, CHW], F16, tag="rm")
                        nc.vector.tensor_tensor(mt[:], q_ps[64:128, :], sinT[:, cs], OP.mult)
                        nc.vector.tensor_tensor(qt[0:64, :], at[:], mt[:], OP.subtract)
                        ab = rpp.tile([64, CHW], F16, tag="rb")
                        nc.vector.tensor_tensor(ab[:], q_ps[64:128, :], cosT[:, cs], OP.mult)
                        mb = rpp.tile([64, CHW], F16, tag="rn")
                        nc.vector.tensor_tensor(mb[:], q_ps[0:64, :], sinT[:, cs], OP.mult)
                        nc.vector.tensor_tensor(qt[64:128, :], ab[:], mb[:], OP.add)
                        q_sb.append(qt)
                        # k
                        k_ps = pmm.tile([128, CHW], F32, tag="mm")
                        for ct in range(CT):
                            nc.tensor.matmul(k_ps[:], wk_r[ct][:, hs], xc[ct][:],
                                             start=(ct == 0), stop=(ct == CT - 1))
                        at2 = rpp.tile([64, CHW], F16, tag="ra")
                        nc.vector.tensor_tensor(at2[:], k_ps[0:64, :], cosT[:, cs], OP.mult)
                        mt2 = rpp.tile([64, CHW], F16, tag="rm")
                        nc.vector.tensor_tensor(mt2[:], k_ps[64:128, :], sinT[:, cs], OP.mult)
                        nc.vector.tensor_tensor(k_all[h][0:64, cs], at2[:], mt2[:], OP.subtract)
                        ab2 = rpp.tile([64, CHW], F16, tag="rb")
                        nc.vector.tensor_tensor(ab2[:], k_ps[64:128, :], cosT[:, cs], OP.mult)
                        mb2 = rpp.tile([64, CHW], F16, tag="rn")
                        nc.vector.tensor_tensor(mb2[:], k_ps[0:64, :], sinT[:, cs], OP.mult)
                        nc.vector.tensor_tensor(k_all[h][64:128, cs], ab2[:], mb2[:], OP.add)
                        # v
                        v_ps = pmm.tile([128, CHW], F32, tag="mm")
                        for ct in range(CT):
                            nc.tensor.matmul(v_ps[:], wv_r[ct][:, hs], xc[ct][:],
                                             start=(ct == 0), stop=(ct == CT - 1))
                        vstg = vstgp.tile([128, CHW], F16, tag="vstg")
                        nc.scalar.copy(vstg[:], v_ps[:])
                        for j in range(CHW // 128):
                            tp = pS.tile([128, 128], F16, tag="S")
                            nc.tensor.transpose(tp[:], vstg[:, j * 128:(j + 1) * 128], ident[:])
                            srow = t0 + j * 128
                            nc.vector.tensor_copy(v_all[h][:, srow:srow + 128], tp[:])

                    # ---- attention ----
                    yT = []
                    n_st = 4 * ch + 4
                    for h in range(HPC):
                        U_ps = pU.tile([128, CHW], F32, tag="U")
                        D_ps = pD.tile([128, CHW], F32, tag="Dn")
                        for st in range(n_st):
                            s_ps = pS.tile([128, CHW], F32, tag="S")
                            nc.tensor.matmul(s_ps[:], k_all[h][:, st * STW:(st + 1) * STW], q_sb[h][:],
                                             start=True, stop=True)
                            e = ep.tile([128, CHW], F16, tag="e")
                            nc.scalar.activation(e[:], s_ps[:], AF.Exp)
                            if st >= 4 * ch:
                                i = st - 4 * ch
                                # keep e[s, t] where t - s - i*128 >= 0, else 0
                                nc.gpsimd.affine_select(
                                    out=e[:], in_=e[:],
                                    compare_op=OP.is_ge, fill=0.0,
                                    base=-(i * STW), channel_multiplier=-1,
                                    pattern=[[1, CHW]],
                                )
                            nc.tensor.matmul(U_ps[:], v_all[h][:, st * STW:(st + 1) * STW], e[:],
                                             start=(st == 0), stop=(st == n_st - 1))
                            nc.tensor.matmul(D_ps[:], ones16[:], e[:],
                                             start=(st == 0), stop=(st == n_st - 1))
                        rD = rdp.tile([128, CHW], F32, tag="rd")
                        nc.vector.reciprocal(rD[:], D_ps[:])
                        yt = yp.tile([128, CHW], F16, tag=f"y{h}")
                        nc.vector.tensor_tensor(yt[:], U_ps[:], rD[:], OP.mult)
                        yT.append(yt)

                    # ---- partial proj for this chunk -> DRAM -> ReduceScatter ----
                    partial = drp.tile([C, CHW], F16, tag="part", name=f"part{g}")
                    for oc in range(CT):
                        o_ps = pmm.tile([128, CHW], F32, tag="mm")
                        nc.tensor.matmul(o_ps[:], wo_r[0][:, oc * 128:(oc + 1) * 128], yT[0][:],
                                         start=True, stop=False)
                        nc.tensor.matmul(o_ps[:], wo_r[1][:, oc * 128:(oc + 1) * 128], yT[1][:],
                                         start=False, stop=True)
                        o_sb = obp.tile([128, CHW], F16, tag="osb")
                        if oc % 2 == 0:
                            nc.scalar.copy(o_sb[:], o_ps[:])
                        else:
                            nc.vector.tensor_copy(o_sb[:], o_ps[:])
                        nc.sync.dma_start(partial[oc * 128:(oc + 1) * 128, :], o_sb[:])
                    rs_out = drp.tile([HPC * D, CHW], F16, tag="rsout", name=f"rsout{g}")
                    nc.gpsimd.collective_compute(
                        "ReduceScatter", mybir.AluOpType.add,
                        replica_groups=[list(range(NC))],
                        ins=[partial.opt()], outs=[rs_out.opt()],
                    )
                    nc.sync.dma_start(outp_d[:, g * CHW:(g + 1) * CHW], rs_out[:])

    nc.compile()
    return nc


def host_prepare(x, conv_w, w_attn, w_proj):
    """Build the per-core consolidated fp16 blob."""
    xf = x.reshape(B * T, C)                           # token-major

    t = np.arange(T, dtype=np.float64)
    inv_freq = 1.0 / (10000.0 ** (np.arange(0, D, 2, dtype=np.float64) / D))
    freqs = np.outer(inv_freq, t)                      # [64, T]
    tabs = np.concatenate([np.cos(freqs), np.sin(freqs)], axis=0).astype(np.float16)

    convw16 = conv_w[:, 0, :].astype(np.float16)       # [2048, 3]
    scale = 1.0 / np.sqrt(np.float32(D))
    in_maps = []
    for c in range(NC):
        tok0 = c * CHW
        xsh = np.zeros((C, XW), dtype=np.float16)
        xsh[:, 2:] = xf[tok0:tok0 + CHW].T.astype(np.float16)
        if c % NCH != 0:                               # halo from previous chunk
            xsh[:, 0:2] = xf[tok0 - 2:tok0].T.astype(np.float16)
        h0 = c * HPC
        rq = slice(h0 * D, (h0 + HPC) * D)
        blob = np.empty((NROWS, 256), dtype=np.float16)
        blob[0:XROWS] = xsh.reshape(XROWS, 256)
        blob[XROWS:AGROWS] = tabs[c * 16:(c + 1) * 16].reshape(TABROWS, 256)
        blob[QKV0:QKV0 + C] = (w_attn[rq, :] * scale).T.astype(np.float16)
        blob[QKV0 + C:QKV0 + 2 * C] = w_attn[C + rq.start:C + rq.stop, :].T.astype(np.float16)
        blob[QKV0 + 2 * C:QKV0 + 3 * C] = w_attn[2 * C + rq.start:2 * C + rq.stop, :].T.astype(np.float16)
        blob[WO0:WO0 + C] = w_proj[:, rq].T.astype(np.float16).reshape(C, 256)
        blob[CW0:NROWS] = convw16.reshape(-1, 256)
        in_maps.append({"blob": blob})
    return in_maps


def host_finish(results):
    outT = np.concatenate([r["outp"] for r in results], axis=0)   # [C, B*T] fp16
    return outT.astype(np.float32).reshape(C, B, T).transpose(1, 2, 0)


_CACHE = {}


def kernel(x, conv_w, w_attn, w_proj):
    x = np.ascontiguousarray(x, dtype=np.float32)
    conv_w = np.ascontiguousarray(conv_w, dtype=np.float32)
    w_attn = np.ascontiguousarray(w_attn, dtype=np.float32)
    w_proj = np.ascontiguousarray(w_proj, dtype=np.float32)
    if "nc" not in _CACHE:
        _CACHE["nc"] = build_program()
    in_maps = host_prepare(x, conv_w, w_attn, w_proj)
    res = bass_utils.run_bass_kernel_spmd(_CACHE["nc"], in_maps, core_ids=list(range(NC)))
    return host_finish(res.results)
